# revision 4
# baseline (speedup 1.0000x reference)
"""ComplexMoE Trainium2 kernel.

Computes, for z_real/z_imag [N, D], gate weights Wg [E, 2D], bg [E], and
per-expert complex weights Wr/Wi [E, D, D]:

    gates = softmax(concat(z_r, z_i) @ Wg.T + bg)            [N, E]
    out_r = sum_e gates[:, e] * (z_r @ Wr_e.T - z_i @ Wi_e.T)
    out_i = sum_e gates[:, e] * (z_i @ Wr_e.T + z_r @ Wi_e.T)
    return stack([out_r, out_i])                             [2, N, D]

Strategy: data-parallel over tokens across 8 NeuronCores (1024 tokens each,
gate + expert weights replicated).  Per expert the complex matmul uses the
3-multiplication Karatsuba form:

    P1 = z_r @ Wr_e.T;  P2 = z_i @ Wi_e.T;  P3 = (z_r+z_i) @ (Wr_e+Wi_e).T
    out_r_e = P1 - P2;  out_i_e = P3 - P1 - P2

with three SBUF accumulators acc_k = sum_e g_e * Pk_e updated by fused
scalar_tensor_tensor ops and a single combine at the end.  Matmuls keep
tokens on PSUM partitions (stationary = z^T chunks, moving = W^T) so the
per-token gate is a per-partition scalar.  All matmul operands are
float32r (full fp32 storage, fast PE mode, ~1e-4 matmul precision).
Tokens are processed in two groups of 512 so the resident z tensors plus
double-buffered weights fit SBUF; weights stream twice.
"""

import sys

try:
    import concourse.bass as bass  # noqa: F401
except ImportError:
    sys.path.insert(0, "/opt/trn_rl_repo")

import numpy as np

import concourse.bass as bass
from concourse import bacc
import concourse.mybir as mybir
from concourse.tile import TileContext
from concourse.bass_utils import run_bass_kernel_spmd

dt = mybir.dt

# ---------------------------------------------------------------- config
N_CORES = 8
N = 8192
D = 1024
E = 8
N_LOC = N // N_CORES            # tokens per core
P = 128                         # partitions
IC = D // P                     # 8 contraction chunks per z tensor
OBLK = 512                      # output-feature block (one PSUM bank)
N_OBLK = D // OBLK              # 2
TOK_GROUPS = 2                  # token groups per core (SBUF residency)
TG_TOK = N_LOC // TOK_GROUPS    # 512 tokens per group
TG_CHUNKS = TG_TOK // P         # 4 token chunks per group

MM_DT = dt.float32r             # matmul dtype on device

TRACE = False                   # set by test harness to capture HW timing
LAST_RESULTS = None             # BassKernelResults of the last run

_BUILT = None


def _build_module():
    nc = bacc.Bacc("TRN2", target_bir_lowering=False, debug=False)

    d_zr = nc.dram_tensor("zrT", [D, N_LOC], MM_DT, kind="ExternalInput").ap()
    d_zi = nc.dram_tensor("ziT", [D, N_LOC], MM_DT, kind="ExternalInput").ap()
    d_zs = nc.dram_tensor("zsT", [D, N_LOC], MM_DT, kind="ExternalInput").ap()
    d_wr = nc.dram_tensor("wrT", [E, D, D], MM_DT, kind="ExternalInput").ap()
    d_wi = nc.dram_tensor("wiT", [E, D, D], MM_DT, kind="ExternalInput").ap()
    d_ws = nc.dram_tensor("wsT", [E, D, D], MM_DT, kind="ExternalInput").ap()
    d_wg = nc.dram_tensor("wgT", [2 * D, E], MM_DT, kind="ExternalInput").ap()
    d_bg = nc.dram_tensor("bgt", [P, E], dt.float32, kind="ExternalInput").ap()
    d_or = nc.dram_tensor("outr", [N_LOC, D], dt.float32, kind="ExternalOutput").ap()
    d_oi = nc.dram_tensor("outi", [N_LOC, D], dt.float32, kind="ExternalOutput").ap()

    AF = mybir.ActivationFunctionType
    ALU = mybir.AluOpType

    with TileContext(nc, trace_sim=False) as tc:
        with (
            tc.tile_pool(name="cst", bufs=1) as cpool,
            tc.tile_pool(name="zres", bufs=1) as zpool,
            tc.tile_pool(name="wmov", bufs=2) as wpool,
            tc.tile_pool(name="accs", bufs=2) as apool,
            tc.tile_pool(name="gate", bufs=2) as gpool,
            tc.tile_pool(name="outs", bufs=3) as opool,
            tc.tile_pool(name="ps_main", bufs=2, space="PSUM") as pspool,
        ):
            # ---- constants: gate weights + bias (resident for whole kernel)
            wg = cpool.tile([P, 2 * IC, E], MM_DT, name="wg")
            nc.sync.dma_start(out=wg[:], in_=d_wg.rearrange("(c p) e -> p c e", p=P))
            bg = cpool.tile([P, E], dt.float32, name="bg")
            nc.sync.dma_start(out=bg[:], in_=d_bg)

            for tg in range(TOK_GROUPS):
                tsl = bass.ts(tg, TG_TOK)
                # ---- resident z^T tensors for this token group: [P, IC, TG_TOK]
                zr = zpool.tile([P, IC, TG_TOK], MM_DT, name="zr", tag="zr")
                zi = zpool.tile([P, IC, TG_TOK], MM_DT, name="zi", tag="zi")
                zs = zpool.tile([P, IC, TG_TOK], MM_DT, name="zs", tag="zs")
                nc.sync.dma_start(
                    out=zr[:], in_=d_zr.rearrange("(c p) n -> p c n", p=P)[:, :, tsl]
                )
                nc.sync.dma_start(
                    out=zi[:], in_=d_zi.rearrange("(c p) n -> p c n", p=P)[:, :, tsl]
                )
                nc.sync.dma_start(
                    out=zs[:], in_=d_zs.rearrange("(c p) n -> p c n", p=P)[:, :, tsl]
                )

                # ---- gates for this group: g_all[:, t, e]
                g_all = gpool.tile(
                    [P, TG_CHUNKS, E], dt.float32, name="g_all", tag="g_all"
                )
                for t in range(TG_CHUNKS):
                    ps_g = pspool.tile([P, E], dt.float32, name="ps_g", tag="p1")
                    for c in range(2 * IC):
                        zsrc = zr if c < IC else zi
                        nc.tensor.matmul(
                            ps_g[:],
                            lhsT=zsrc[:, c % IC, bass.ts(t, P)],
                            rhs=wg[:, c, :],
                            start=(c == 0),
                            stop=(c == 2 * IC - 1),
                        )
                    lg = gpool.tile([P, E], dt.float32, name="lg", tag="lg")
                    nc.vector.tensor_add(out=lg[:], in0=ps_g[:], in1=bg[:])
                    u = gpool.tile([P, E], dt.float32, name="u", tag="u")
                    nc.scalar.activation(u[:], lg[:], AF.Exp)
                    s = gpool.tile([P, 1], dt.float32, name="s", tag="s")
                    nc.vector.tensor_reduce(
                        s[:], u[:], axis=mybir.AxisListType.X, op=ALU.add
                    )
                    r = gpool.tile([P, 1], dt.float32, name="r", tag="r")
                    nc.vector.reciprocal(r[:], s[:])
                    nc.vector.tensor_scalar_mul(g_all[:, t, :], u[:], r[:])

                # ---- main loop over output blocks and experts
                for ob in range(N_OBLK):
                    acc1 = apool.tile(
                        [P, TG_CHUNKS, OBLK], dt.float32, name="acc1", tag="acc1"
                    )
                    acc2 = apool.tile(
                        [P, TG_CHUNKS, OBLK], dt.float32, name="acc2", tag="acc2"
                    )
                    acc3 = apool.tile(
                        [P, TG_CHUNKS, OBLK], dt.float32, name="acc3", tag="acc3"
                    )
                    for e in range(E):
                        wr = wpool.tile([P, IC, OBLK], MM_DT, name="wr", tag="wr")
                        wi = wpool.tile([P, IC, OBLK], MM_DT, name="wi", tag="wi")
                        ws = wpool.tile([P, IC, OBLK], MM_DT, name="ws", tag="ws")
                        osl = bass.ts(ob, OBLK)
                        nc.sync.dma_start(
                            out=wr[:],
                            in_=d_wr[e].rearrange("(c p) o -> p c o", p=P)[:, :, osl],
                        )
                        nc.sync.dma_start(
                            out=wi[:],
                            in_=d_wi[e].rearrange("(c p) o -> p c o", p=P)[:, :, osl],
                        )
                        nc.sync.dma_start(
                            out=ws[:],
                            in_=d_ws[e].rearrange("(c p) o -> p c o", p=P)[:, :, osl],
                        )

                        for t in range(TG_CHUNKS):
                            ps = []
                            for name, zt, wt in (
                                ("p1", zr, wr),
                                ("p2", zi, wi),
                                ("p3", zs, ws),
                            ):
                                pk = pspool.tile(
                                    [P, OBLK], dt.float32, name=name, tag=name
                                )
                                for c in range(IC):
                                    nc.tensor.matmul(
                                        pk[:],
                                        lhsT=zt[:, c, bass.ts(t, P)],
                                        rhs=wt[:, c, :],
                                        start=(c == 0),
                                        stop=(c == IC - 1),
                                    )
                                ps.append(pk)
                            gcol = g_all[:, t, e : e + 1]
                            for pk, acc in zip(ps, (acc1, acc2, acc3)):
                                if e == 0:
                                    nc.vector.tensor_scalar_mul(
                                        acc[:, t, :], pk[:], gcol
                                    )
                                else:
                                    nc.vector.scalar_tensor_tensor(
                                        out=acc[:, t, :],
                                        in0=pk[:],
                                        scalar=gcol,
                                        in1=acc[:, t, :],
                                        op0=ALU.mult,
                                        op1=ALU.add,
                                    )

                    # final combine + store: rows tg*TG_TOK + t*P
                    for t in range(TG_CHUNKS):
                        rsl = bass.ds(tg * TG_TOK + t * P, P)
                        osl = bass.ts(ob, OBLK)
                        o_r = opool.tile([P, OBLK], dt.float32, name="o_r", tag="o_r")
                        nc.vector.tensor_sub(
                            out=o_r[:], in0=acc1[:, t, :], in1=acc2[:, t, :]
                        )
                        nc.sync.dma_start(out=d_or[rsl, osl], in_=o_r[:])
                        o_i = opool.tile([P, OBLK], dt.float32, name="o_i", tag="o_i")
                        nc.vector.tensor_sub(
                            out=o_i[:], in0=acc3[:, t, :], in1=acc1[:, t, :]
                        )
                        nc.vector.tensor_sub(
                            out=o_i[:], in0=o_i[:], in1=acc2[:, t, :]
                        )
                        nc.sync.dma_start(out=d_oi[rsl, osl], in_=o_i[:])

    nc.compile()
    return nc


def kernel(z_real, z_imag, Wg, bg, Wr, Wi):
    global _BUILT, LAST_RESULTS
    assert z_real.shape == (N, D) and z_imag.shape == (N, D)

    if _BUILT is None:
        _BUILT = _build_module()
    nc = _BUILT

    # ---- host-side prep (layout only; f32 bits pass through as float32r)
    f32 = np.float32
    wrT = np.ascontiguousarray(Wr.transpose(0, 2, 1)).astype(f32)   # [E, i, o]
    wiT = np.ascontiguousarray(Wi.transpose(0, 2, 1)).astype(f32)
    wsT = wrT + wiT
    wgT = np.ascontiguousarray(np.asarray(Wg, f32).T)               # [2D, E]
    bgt = np.tile(np.asarray(bg, f32).reshape(1, E), (P, 1))

    zrT = np.ascontiguousarray(np.asarray(z_real, f32).T)           # [D, N]
    ziT = np.ascontiguousarray(np.asarray(z_imag, f32).T)
    zsT = zrT + ziT

    in_maps = []
    for c in range(N_CORES):
        sl = slice(c * N_LOC, (c + 1) * N_LOC)
        in_maps.append(
            {
                "zrT": np.ascontiguousarray(zrT[:, sl]),
                "ziT": np.ascontiguousarray(ziT[:, sl]),
                "zsT": np.ascontiguousarray(zsT[:, sl]),
                "wrT": wrT,
                "wiT": wiT,
                "wsT": wsT,
                "wgT": wgT,
                "bgt": bgt,
            }
        )

    res = run_bass_kernel_spmd(
        nc, in_maps, core_ids=list(range(N_CORES)), trace=TRACE
    )
    LAST_RESULTS = res

    out = np.empty((2, N, D), dtype=np.float32)
    for c in range(N_CORES):
        sl = slice(c * N_LOC, (c + 1) * N_LOC)
        out[0, sl] = res.results[c]["outr"]
        out[1, sl] = res.results[c]["outi"]
    return out


# revision 9
# speedup vs baseline: 1.0318x; 1.0318x over previous
"""ComplexMoE Trainium2 kernel.

Computes, for z_real/z_imag [N, D], gate weights Wg [E, 2D], bg [E], and
per-expert complex weights Wr/Wi [E, D, D]:

    gates = softmax(concat(z_r, z_i) @ Wg.T + bg)            [N, E]
    out_r = sum_e gates[:, e] * (z_r @ Wr_e.T - z_i @ Wi_e.T)
    out_i = sum_e gates[:, e] * (z_i @ Wr_e.T + z_r @ Wi_e.T)
    return stack([out_r, out_i])                             [2, N, D]

Strategy: data-parallel over tokens across 8 NeuronCores (1024 tokens each,
gate + expert weights replicated).  Per expert the complex matmul uses the
3-multiplication Karatsuba form:

    P1 = z_r @ Wr_e.T;  P2 = z_i @ Wi_e.T;  P3 = (z_r+z_i) @ (Wr_e+Wi_e).T
    out_r_e = P1 - P2;  out_i_e = P3 - P1 - P2

with three SBUF accumulators acc_k = sum_e g_e * Pk_e updated by fused
scalar_tensor_tensor ops and a single combine at the end.  Matmuls keep
tokens on PSUM partitions (stationary = z^T chunks, moving = W^T) so the
per-token gate is a per-partition scalar.  All matmul operands are
float32r (full fp32 storage, fast PE mode, ~1e-4 matmul precision).
Tokens are processed in two groups of 512 so the resident z tensors plus
double-buffered weights fit SBUF; weights stream twice.
"""

import sys

try:
    import concourse.bass as bass  # noqa: F401
except ImportError:
    sys.path.insert(0, "/opt/trn_rl_repo")

import numpy as np

import concourse.bass as bass
from concourse import bacc
import concourse.mybir as mybir
from concourse.tile import TileContext
from concourse.bass_utils import run_bass_kernel_spmd

dt = mybir.dt

# ---------------------------------------------------------------- config
N_CORES = 8
N = 8192
D = 1024
E = 8
N_LOC = N // N_CORES            # tokens per core
P = 128                         # partitions
IC = D // P                     # 8 contraction chunks per z tensor
OBLK = 512                      # output-feature block (one PSUM bank)
N_OBLK = D // OBLK              # 2
TOK_GROUPS = 2                  # token groups per core (SBUF residency)
TG_TOK = N_LOC // TOK_GROUPS    # 512 tokens per group
TG_CHUNKS = TG_TOK // P         # 4 token chunks per group

MM_DT = dt.float32r             # matmul dtype on device

TRACE = False                   # set by test harness to capture HW timing
LAST_RESULTS = None             # BassKernelResults of the last run

_BUILT = None


def _build_module():
    nc = bacc.Bacc("TRN2", target_bir_lowering=False, debug=False)

    d_zr = nc.dram_tensor("zrT", [D, N_LOC], MM_DT, kind="ExternalInput").ap()
    d_zi = nc.dram_tensor("ziT", [D, N_LOC], MM_DT, kind="ExternalInput").ap()
    d_zs = nc.dram_tensor("zsT", [D, N_LOC], MM_DT, kind="ExternalInput").ap()
    d_wr = nc.dram_tensor("wrT", [E, D, D], MM_DT, kind="ExternalInput").ap()
    d_wi = nc.dram_tensor("wiT", [E, D, D], MM_DT, kind="ExternalInput").ap()
    d_ws = nc.dram_tensor("wsT", [E, D, D], MM_DT, kind="ExternalInput").ap()
    d_wg = nc.dram_tensor("wgT", [2 * D, E], MM_DT, kind="ExternalInput").ap()
    d_bg = nc.dram_tensor("bgc", [E, 1], dt.float32, kind="ExternalInput").ap()
    d_or = nc.dram_tensor("outr", [N_LOC, D], dt.float32, kind="ExternalOutput").ap()
    d_oi = nc.dram_tensor("outi", [N_LOC, D], dt.float32, kind="ExternalOutput").ap()

    AF = mybir.ActivationFunctionType
    ALU = mybir.AluOpType

    with TileContext(nc, trace_sim=False) as tc:
        with (
            tc.tile_pool(name="cst", bufs=1) as cpool,
            tc.tile_pool(name="zres", bufs=1) as zpool,
            tc.tile_pool(name="wmov", bufs=2) as wpool,
            tc.tile_pool(name="accs", bufs=2) as apool,
            tc.tile_pool(name="gate", bufs=2) as gpool,
            tc.tile_pool(name="outs", bufs=2) as opool,
            tc.tile_pool(name="ps_main", bufs=2, space="PSUM") as pspool,
        ):
            # ---- constants: gate weights + bias (resident for whole kernel)
            wg = cpool.tile([P, 2 * IC, E], MM_DT, name="wg")
            nc.sync.dma_start(out=wg[:], in_=d_wg.rearrange("(c p) e -> p c e", p=P))
            bgc = cpool.tile([E, 1], dt.float32, name="bgc")
            nc.sync.dma_start(out=bgc[:], in_=d_bg)
            ident = cpool.tile([E, E], dt.float32, name="ident")
            from concourse.masks import make_identity

            make_identity(nc, ident[:])

            for tg in range(TOK_GROUPS):
                tsl = bass.ts(tg, TG_TOK)
                # ---- resident z^T tensors for this token group: [P, IC, TG_TOK]
                zr = zpool.tile([P, IC, TG_TOK], MM_DT, name="zr", tag="zr")
                zi = zpool.tile([P, IC, TG_TOK], MM_DT, name="zi", tag="zi")
                zs = zpool.tile([P, IC, TG_TOK], MM_DT, name="zs", tag="zs")
                zsrc_dram = {"zr": d_zr, "zi": d_zi, "zs": d_zs}
                for nm, zt in (("zr", zr), ("zi", zi), ("zs", zs)):
                    src = zsrc_dram[nm].rearrange("(c p) n -> p c n", p=P)
                    for c in range(IC):
                        nc.sync.dma_start(out=zt[:, c, :], in_=src[:, c, tsl])

                # ---- gates, weight-stationary: logits^T [E, TG_TOK] in PSUM
                lgT = pspool.tile([E, TG_TOK], dt.float32, name="lgT", tag="pg")
                for c in range(2 * IC):
                    zsrc = zr if c < IC else zi
                    nc.tensor.matmul(
                        lgT[:],
                        lhsT=wg[:, c, :],
                        rhs=zsrc[:, c % IC, :],
                        start=(c == 0),
                        stop=(c == 2 * IC - 1),
                    )
                # u^T = exp(logits^T + bg) on ACT, then transpose chunks back
                uT = gpool.tile([E, TG_TOK], dt.float32, name="uT", tag="uT")
                nc.scalar.activation(uT[:], lgT[:], AF.Exp, bias=bgc[:])
                g_all = gpool.tile(
                    [P, TG_CHUNKS, E], dt.float32, name="g_all", tag="g_all"
                )
                for t in range(TG_CHUNKS):
                    tp = pspool.tile([P, E], dt.float32, name="tp", tag="pg")
                    nc.tensor.transpose(
                        tp[:], in_=uT[:, bass.ts(t, P)], identity=ident[:]
                    )
                    s = gpool.tile([P, 1], dt.float32, name="s", tag="s")
                    nc.vector.tensor_reduce(
                        s[:], tp[:], axis=mybir.AxisListType.X, op=ALU.add
                    )
                    r = gpool.tile([P, 1], dt.float32, name="r", tag="r")
                    nc.vector.reciprocal(r[:], s[:])
                    nc.vector.tensor_scalar_mul(g_all[:, t, :], tp[:], r[:])

                # ---- main loop over output blocks and experts
                for ob in range(N_OBLK):
                    acc1 = apool.tile(
                        [P, TG_CHUNKS, OBLK], dt.float32, name="acc1", tag="acc1"
                    )
                    acc2 = apool.tile(
                        [P, TG_CHUNKS, OBLK], dt.float32, name="acc2", tag="acc2"
                    )
                    acc3 = apool.tile(
                        [P, TG_CHUNKS, OBLK], dt.float32, name="acc3", tag="acc3"
                    )
                    for e in range(E):
                        wr = wpool.tile([P, IC, OBLK], MM_DT, name="wr", tag="wr")
                        wi = wpool.tile([P, IC, OBLK], MM_DT, name="wi", tag="wi")
                        ws = wpool.tile([P, IC, OBLK], MM_DT, name="ws", tag="ws")
                        osl = bass.ts(ob, OBLK)
                        nc.sync.dma_start(
                            out=wr[:],
                            in_=d_wr[e].rearrange("(c p) o -> p c o", p=P)[:, :, osl],
                        )
                        nc.sync.dma_start(
                            out=wi[:],
                            in_=d_wi[e].rearrange("(c p) o -> p c o", p=P)[:, :, osl],
                        )
                        nc.sync.dma_start(
                            out=ws[:],
                            in_=d_ws[e].rearrange("(c p) o -> p c o", p=P)[:, :, osl],
                        )

                        for t in range(TG_CHUNKS):
                            ps = []
                            for name, zt, wt in (
                                ("p1", zr, wr),
                                ("p2", zi, wi),
                                ("p3", zs, ws),
                            ):
                                pk = pspool.tile(
                                    [P, OBLK], dt.float32, name=name, tag=name
                                )
                                for c in range(IC):
                                    nc.tensor.matmul(
                                        pk[:],
                                        lhsT=zt[:, c, bass.ts(t, P)],
                                        rhs=wt[:, c, :],
                                        start=(c == 0),
                                        stop=(c == IC - 1),
                                    )
                                ps.append(pk)
                            gcol = g_all[:, t, e : e + 1]
                            for pk, acc in zip(ps, (acc1, acc2, acc3)):
                                if e == 0:
                                    nc.vector.tensor_scalar_mul(
                                        acc[:, t, :], pk[:], gcol
                                    )
                                else:
                                    nc.vector.scalar_tensor_tensor(
                                        out=acc[:, t, :],
                                        in0=pk[:],
                                        scalar=gcol,
                                        in1=acc[:, t, :],
                                        op0=ALU.mult,
                                        op1=ALU.add,
                                    )

                    # final combine + store: rows tg*TG_TOK + t*P
                    for t in range(TG_CHUNKS):
                        rsl = bass.ds(tg * TG_TOK + t * P, P)
                        osl = bass.ts(ob, OBLK)
                        o_r = opool.tile([P, OBLK], dt.float32, name="o_r", tag="o_r")
                        nc.vector.tensor_sub(
                            out=o_r[:], in0=acc1[:, t, :], in1=acc2[:, t, :]
                        )
                        nc.sync.dma_start(out=d_or[rsl, osl], in_=o_r[:])
                        o_i = opool.tile([P, OBLK], dt.float32, name="o_i", tag="o_i")
                        nc.vector.tensor_sub(
                            out=o_i[:], in0=acc3[:, t, :], in1=acc1[:, t, :]
                        )
                        nc.vector.tensor_sub(
                            out=o_i[:], in0=o_i[:], in1=acc2[:, t, :]
                        )
                        nc.sync.dma_start(out=d_oi[rsl, osl], in_=o_i[:])

    nc.compile()
    return nc


def kernel(z_real, z_imag, Wg, bg, Wr, Wi):
    global _BUILT, LAST_RESULTS
    assert z_real.shape == (N, D) and z_imag.shape == (N, D)

    if _BUILT is None:
        _BUILT = _build_module()
    nc = _BUILT

    # ---- host-side prep (layout only; f32 bits pass through as float32r)
    f32 = np.float32
    wrT = np.ascontiguousarray(Wr.transpose(0, 2, 1)).astype(f32)   # [E, i, o]
    wiT = np.ascontiguousarray(Wi.transpose(0, 2, 1)).astype(f32)
    wsT = wrT + wiT
    wgT = np.ascontiguousarray(np.asarray(Wg, f32).T)               # [2D, E]
    bgc = np.ascontiguousarray(np.asarray(bg, f32).reshape(E, 1))

    zrT = np.ascontiguousarray(np.asarray(z_real, f32).T)           # [D, N]
    ziT = np.ascontiguousarray(np.asarray(z_imag, f32).T)
    zsT = zrT + ziT

    in_maps = []
    for c in range(N_CORES):
        sl = slice(c * N_LOC, (c + 1) * N_LOC)
        in_maps.append(
            {
                "zrT": np.ascontiguousarray(zrT[:, sl]),
                "ziT": np.ascontiguousarray(ziT[:, sl]),
                "zsT": np.ascontiguousarray(zsT[:, sl]),
                "wrT": wrT,
                "wiT": wiT,
                "wsT": wsT,
                "wgT": wgT,
                "bgc": bgc,
            }
        )

    res = run_bass_kernel_spmd(
        nc, in_maps, core_ids=list(range(N_CORES)), trace=TRACE
    )
    LAST_RESULTS = res

    out = np.empty((2, N, D), dtype=np.float32)
    for c in range(N_CORES):
        sl = slice(c * N_LOC, (c + 1) * N_LOC)
        out[0, sl] = res.results[c]["outr"]
        out[1, sl] = res.results[c]["outi"]
    return out
